# revision 31
# baseline (speedup 1.0000x reference)
"""Trainium2 Bass kernel: multi-head attention block (B=4, N=2048, C=1024, H=16).

Sharding: 8 cores = (batch b in 0..3) x (head-group hg in 0..1, 8 heads each).
Each core computes qkv for its heads, full attention for its heads over its
batch, and a partial projection (its 512 rows of W_proj). Host sums the two
partials per batch and adds the effective bias.

Device layout (all matmuls bf16 inputs, fp32 PSUM accumulate):
  - q,k produced in transposed layout qkT[dim, token] so S^T = k-chunk.T @ q^T
    needs no on-chip transposes.
  - v produced in natural layout [token, 65*h] with a trailing ones column per
    head, so the O matmul lhsT=[v|ones] gives row 64 = softmax denominator and
    rows 0..63 = unnormalized o^T in one PSUM accumulation chain.
  - head PAIRS: the two heads' S matmuls sit at base partitions 0/64 (distinct
    PE row groups) and run concurrently.

v3 perf structure (baseline 364us -> v2 353.5us -> this):
  - unified big-PSUM pool (bufs=3, 6 banks) for v/qk/S/proj accumulators: the
    attention S ring is 3 deep, so the inner loop is throughput-bound on the
    PE instead of latency-bound on exp PSUM release.
  - attention emitted in 4j GROUPS [S,S,S, O-batch-of-8, S] with a pend queue
    that rolls ACROSS i4 blocks (lag ~8 chunks): long same-shape O runs cut
    the ~100ns 64<->128-row-mode transition tax from ~17 to ~8 per block and
    remove the block-boundary pipeline drains; the batch sits right before
    the st-ring-gated 4th S so the PE chews O work during the exp wait.
  - host passes all inputs PRE-REARRANGED partition-major ([128, ...] C-order)
    so every load DMA has 2-16KB contiguous lines instead of the 256B-1KB
    strided runs of the (c p)->p c rearrange (those capped HBM at ~240GB/s
    and pushed the first v matmul to 14us).  wv + the first xT token block
    land first; the first v chain starts ~4us earlier.
  - bias algebra: v-bias folded into the host-side output bias
    (b_eff = b_proj + b_v @ W_proj, exact because softmax rows sum to 1);
    k-bias dropped entirely (softmax is invariant to per-query shifts).
  - exp split: ScalarE exact exp (A) / VectorE Schraudolph bit-trick (D) in
    strict alternation ADAD...AA (9A/7D) so consecutive chunks land on
    different engines; ~2% rms on D chunks, total rel err ~1.5e-2 vs 2e-2.
  - PE warm-up matmuls during the initial DMA wait (HAM clock gate).
"""

import os
from contextlib import ExitStack

import numpy as np
import ml_dtypes

import concourse.bass as bass
import concourse.tile as tile
from concourse import bacc, mybir
from concourse.bass import ds, ts
from concourse.bass_utils import run_bass_kernel_spmd

try:  # without the NTFF hook module, a stray BASS_TRACE=1 would crash the run
    from antenv.axon_hooks import get_axon_ntff_profile_hook  # noqa: F401
except ImportError:
    os.environ.setdefault("BASS_NEVER_TRACE", "1")

BF16 = mybir.dt.bfloat16
F32 = mybir.dt.float32
NP_BF16 = ml_dtypes.bfloat16

LOG2E = float(np.log2(np.e))
SHIFT = 0.0  # bf16 exp needs no range shift (max logit 8.08 -> e^8 fits)
A16 = (2.0**7) * LOG2E / 8.0
B16 = 127.0 * 2**7 - (2.0**7) * LOG2E * SHIFT - 5.5
I16 = mybir.dt.int16
# per-i4-block chunk pattern: A = ScalarE exact exp, D = VectorE Schraudolph
PATTERN = "AADADADADADADAAA"

B, N, C = 4, 2048, 1024
H, D = 16, 64
HPC = 8            # heads per core
CD = HPC * D       # 512 local qkv dims per core
E = D + 1          # 65: 64 v dims + ones column

LAST_RESULTS = None  # stash for test harness (exec_time_ns, trace paths)


def _build_program():
    nc = bacc.Bacc("TRN2", target_bir_lowering=False, debug=False)

    # all inputs partition-major, pre-rearranged on host for contiguous DMA
    xT_d = nc.dram_tensor("xT", [128, 8, N], BF16, kind="ExternalInput").ap()
    wqk_d = nc.dram_tensor("wqk", [128, 8, 2 * CD], BF16, kind="ExternalInput").ap()
    wv_d = nc.dram_tensor("wv", [128, 8, CD], BF16, kind="ExternalInput").ap()
    bq_d = nc.dram_tensor("bq", [128, 4], F32, kind="ExternalInput").ap()
    wp_d = nc.dram_tensor("wp", [128, 4, C], BF16, kind="ExternalInput").ap()
    out_d = nc.dram_tensor("out", [N, C], F32, kind="ExternalOutput").ap()

    with tile.TileContext(nc) as tc, ExitStack() as ctx:
        singles = ctx.enter_context(tc.tile_pool(name="singles", bufs=1))
        # one big-PSUM pool: 3 x [128,1024]f32 slots (6 banks) shared by the
        # warmup/v/qk/S/proj accumulators -> 3-deep S ring in attention
        ps_pool = ctx.enter_context(tc.tile_pool(name="ps", bufs=3, space="PSUM"))
        ot_pool = ctx.enter_context(tc.tile_pool(name="ot", bufs=2, space="PSUM"))
        exp_pool = ctx.enter_context(tc.tile_pool(name="expp", bufs=13))
        misc = ctx.enter_context(tc.tile_pool(name="misc", bufs=4))
        ob_pool = ctx.enter_context(tc.tile_pool(name="ob", bufs=4))

        # Persistent SBUF tensors, chunk-major: [partition, chunk, free].
        xT_sb = singles.tile([128, 8, N], BF16)        # x^T   [c, token]
        wqk_sb = singles.tile([128, 8, 2 * CD], BF16)  # W_qk  [c, m]
        wv_sb = singles.tile([128, 8, CD], BF16)       # W_v   [c, n]
        bq_sb = singles.tile([128, 4], F32)
        wp_sb = singles.tile([128, 4, C], BF16)        # W_proj [hd, n]
        qkT_sb = singles.tile([128, 8, N], BF16)       # chunks 0..3 = q, 4..7 = k
        v_sb = singles.tile([128, 16, HPC * E], BF16)  # [token-in-chunk, tchunk, h*(64+1)]
        oT_sb = singles.tile([128, 4, N], BF16)        # o^T, proj lhsT layout
        nbias_sb = singles.tile([128, 1], F32)         # -SHIFT bias for ACT exp

        # DMA priority order = emission order, on TWO HWDGE rings in parallel:
        # the scalar ring issues the weights (wv first -- the v phase blocks
        # on it), the sync ring streams xT in 256-token pieces (2KB
        # contiguous per partition) in v-chunk consumption order.
        nc.scalar.dma_start(wv_sb, wv_d)
        nc.scalar.dma_start(wqk_sb, wqk_d)
        nc.scalar.dma_start(bq_sb, bq_d)
        nc.scalar.dma_start(wp_sb, wp_d)
        # xT pieces: contiguous 1KB runs on BOTH the DRAM and SBUF side,
        # token-quarter-major so the first v chunks unblock earliest
        for tq in range(4):
            for kc in range(8):
                nc.sync.dma_start(
                    xT_sb[:, kc, ds(tq * 512, 512)], xT_d[:, kc, ds(tq * 512, 512)]
                )
        nc.vector.memset(nbias_sb, -SHIFT)

        # dummy exp: pulls the ~2.7us ACT table load into the DMA/v ramp
        # instead of the first real activation on block (0,0)'s critical path
        warm = misc.tile([128, 1], BF16, tag="warm")
        nc.scalar.activation(
            warm, nbias_sb, mybir.ActivationFunctionType.Exp
        )

        # PE warm-up: matmuls on a memset tile during the initial DMA wait
        # flip the HAM clock gate to 8/8 before the first v chunk.
        warm_in = misc.tile([128, 256], BF16, tag="warmin")
        nc.vector.memset(warm_in, 0.0)
        warm_ps = ps_pool.tile([128, 1024], F32, tag="st", name="warmps")
        for _ in range(40):
            nc.tensor.matmul(
                warm_ps[:, 0:256], warm_in[:, 0:128], warm_in, start=True, stop=True
            )

        # ones columns for all 16 v chunks in one strided memset
        nc.vector.memset(
            v_sb.rearrange("p t (h e) -> p t h e", e=E)[:, :, :, D : D + 1], 1.0
        )

        # v natural layout; bias is folded into the host-side output bias.
        def emit_v_chunk(t):
            ps = ps_pool.tile([128, 512], F32, tag="st", name="vps")
            for kc in range(8):
                nc.tensor.matmul(
                    ps,
                    xT_sb[:, kc, ts(t, 128)],
                    wv_sb[:, kc, :],
                    start=(kc == 0),
                    stop=(kc == 7),
                )
            vv = v_sb[:, t].rearrange("p (h e) -> p h e", e=E)
            nc.vector.tensor_copy(vv[:, :, 0:D], ps.rearrange("p (h d) -> p h d", d=D))

        for t in range(16):
            emit_v_chunk(t)

        # qk chunk m: q chunks (m<4) get the q bias on evacuation; k chunks
        # (m>=4) need no bias (softmax is invariant to per-query shifts).
        def emit_qk_chunk(m):
            for i4q in range(4):
                ps = ps_pool.tile([128, 512], F32, tag="st", name="qkps")
                for kc in range(8):
                    nc.tensor.matmul(
                        ps,
                        wqk_sb[:, kc, ts(m, 128)],
                        xT_sb[:, kc, ds(i4q * 512, 512)],
                        start=(kc == 0),
                        stop=(kc == 7),
                    )
                if m < 4:
                    nc.vector.tensor_scalar_add(
                        qkT_sb[:, m, ds(i4q * 512, 512)], ps, bq_sb[:, ds(m, 1)]
                    )
                else:
                    nc.vector.tensor_copy(qkT_sb[:, m, ds(i4q * 512, 512)], ps)

        def emit_proj(t):
            for nh in range(2):
                pp = ps_pool.tile([128, 512], F32, tag="st", name="pp")
                for hc in range(4):
                    nc.tensor.matmul(
                        pp,
                        oT_sb[:, hc, ts(t, 128)],
                        wp_sb[:, hc, ds(nh * 512, 512)],
                        start=(hc == 0),
                        stop=(hc == 3),
                    )
                ob = ob_pool.tile([128, 512], F32)
                nc.vector.tensor_copy(ob, pp)
                nc.sync.dma_start(out_d[ts(t, 128), ds(nh * 512, 512)], ob)

        # Phase 2: per head PAIR. The two heads' S matmuls target different PE
        # row groups (base partitions 0 / 64) so back-to-back issue runs them
        # concurrently; both write one [128, 1024] st tile and share one exp.
        for p in range(4):
            qA, qB = qkT_sb[0:64, p], qkT_sb[64:128, p]
            kA, kB = qkT_sb[0:64, 4 + p], qkT_sb[64:128, 4 + p]
            hA, hB = 2 * p, 2 * p + 1
            emit_qk_chunk(p)
            emit_qk_chunk(4 + p)

            def emit_evac(i4, ot_list):
                i0 = i4 * 512
                for hp_, ot in ot_list:
                    # Copy the whole accumulator out first: frees the PSUM
                    # slot fast; same DVE cost as one row (partitions are
                    # parallel).  MUST stay on DVE: on ScalarE it head-of-line
                    # blocks the exp stream (waits for the whole O chain).
                    otc = misc.tile([65, 512], F32, tag="otc")
                    nc.vector.tensor_copy(otc, ot)
                    # Softmax denominators: lane-scatter so reciprocal runs on
                    # 128 lanes x 4 elems instead of 1 lane x 512
                    s_t = misc.tile([128, 4], F32, tag="sct")
                    nc.sync.dma_start(s_t, otc[64:65])
                    r_t = misc.tile([128, 4], F32, tag="rct")
                    nc.vector.reciprocal(r_t, s_t)
                    rec0 = misc.tile([1, 512], F32, tag="rec0")
                    nc.sync.dma_start(rec0, r_t)
                    recb = misc.tile([64, 512], F32, tag="recb")
                    nc.gpsimd.partition_broadcast(recb, rec0)
                    tmp = misc.tile([64, 512], BF16, tag="tmp")
                    nc.vector.tensor_mul(tmp, otc[0:64], recb)
                    nc.sync.dma_start(oT_sb[hp_ : hp_ + 64, p, ds(i0, 512)], tmp)

            for i4 in range(4):
                i0 = i4 * 512
                otA = ot_pool.tile([65, 512], F32, tag="ot")
                otB = ot_pool.tile([65, 512], F32, tag="ot")
                pend = []

                def emit_o(batch):
                    # uniform run of 4 O matmuls (2 chunks x 2 heads): longer
                    # same-shape runs halve the PE mode-switch tax
                    for ot, h in ((otA, hA), (otB, hB)):
                        for j, ex in batch:
                            vvj = v_sb[:, j].rearrange("p (h e) -> p h e", e=E)
                            nc.tensor.matmul(
                                ot,
                                vvj[:, h],
                                ex[:, ds(512 * (h % 2), 512)],
                                start=(j == 0),
                                stop=(j == 15),
                            )

                for j in range(16):
                    st = ps_pool.tile([128, 1024], F32, tag="st", name="st")
                    nc.tensor.matmul(
                        st[:, 0:512], kA[:, ts(j, 128)], qA[:, ds(i0, 512)],
                        start=True, stop=True,
                    )
                    nc.tensor.matmul(
                        st[:, 512:1024], kB[:, ts(j, 128)], qB[:, ds(i0, 512)],
                        start=True, stop=True,
                    )
                    ex = exp_pool.tile([128, 1024], BF16)
                    if PATTERN[j] == "A":
                        nc.scalar.activation(
                            ex, st, mybir.ActivationFunctionType.Exp,
                            scale=float(D) ** -0.5,
                        )
                    else:
                        nc.vector.tensor_scalar(
                            ex.bitcast(I16), st, A16, B16,
                            mybir.AluOpType.mult, mybir.AluOpType.add,
                        )
                    pend.append((j, ex))
                    # lag-3: emit O only for exps issued >=3 chunks ago, so
                    # the PE never head-of-line blocks on a fresh exp
                    if len(pend) >= 5:
                        emit_o([pend.pop(0), pend.pop(0)])
                while pend:
                    emit_o([pend.pop(0), pend.pop(0)] if len(pend) >= 2 else [pend.pop(0)])
                emit_evac(i4, ((0, otA), (64, otB)))

        for t in range(16):
            emit_proj(t)

    nc.compile()
    return nc


_PROGRAM = None


def _relayout_pcm(w, nchunk):
    # [nchunk*128, M] -> [128, nchunk, M] partition-major contiguous
    m = w.shape[1]
    return np.ascontiguousarray(
        w.reshape(nchunk, 128, m).transpose(1, 0, 2)
    ).astype(NP_BF16)


def kernel(x, W_qkv, b_qkv, W_proj, b_proj):
    global _PROGRAM, LAST_RESULTS
    x = np.asarray(x, dtype=np.float32)
    W_qkv = np.asarray(W_qkv, dtype=np.float32)
    b_qkv = np.asarray(b_qkv, dtype=np.float32)
    W_proj = np.asarray(W_proj, dtype=np.float32)
    b_proj = np.asarray(b_proj, dtype=np.float32)

    if _PROGRAM is None:
        _PROGRAM = _build_program()
    nc = _PROGRAM

    # effective output bias: v-bias flows through softmax (rows sum to 1)
    # straight into the projection
    b_eff = b_proj + b_qkv[2 * C :] @ W_proj

    in_maps = []
    for core in range(8):
        b, hg = core // 2, core % 2
        h0 = hg * HPC
        sl = slice(h0 * D, h0 * D + CD)
        wq = W_qkv[:, 0 * C :][:, sl]
        wk = W_qkv[:, 1 * C :][:, sl]
        wv = W_qkv[:, 2 * C :][:, sl]
        bq = b_qkv[0 * C :][sl]
        # xT: [C, N] -> [128p, 8kc, 2048t] partition-major contiguous
        xT = np.ascontiguousarray(
            x[b].T.reshape(8, 128, N).transpose(1, 0, 2)
        ).astype(NP_BF16)
        in_maps.append(
            {
                "xT": xT,
                "wqk": _relayout_pcm(np.concatenate([wq, wk], axis=1), 8),
                "wv": _relayout_pcm(np.ascontiguousarray(wv), 8),
                "bq": bq.reshape(4, 128).T.astype(np.float32).copy(),
                "wp": _relayout_pcm(np.ascontiguousarray(W_proj[sl, :]), 4),
            }
        )

    res = run_bass_kernel_spmd(nc, in_maps, list(range(8)))
    LAST_RESULTS = res
    out = np.empty((B, N, C), dtype=np.float32)
    for b in range(B):
        out[b] = (
            res.results[2 * b]["out"].astype(np.float32)
            + res.results[2 * b + 1]["out"].astype(np.float32)
            + b_eff[None, :]
        )
    return out


# revision 32
# speedup vs baseline: 1.0043x; 1.0043x over previous
"""Trainium2 Bass kernel: multi-head attention block (B=4, N=2048, C=1024, H=16).

Sharding: 8 cores = (batch b in 0..3) x (head-group hg in 0..1, 8 heads each).
Each core computes qkv for its heads, full attention for its heads over its
batch, and a partial projection (its 512 rows of W_proj). Host sums the two
partials per batch and adds the effective bias.

Device layout (all matmuls bf16 inputs, fp32 PSUM accumulate):
  - q,k produced in transposed layout qkT[dim, token] so S^T = k-chunk.T @ q^T
    needs no on-chip transposes.
  - v produced in natural layout [token, 65*h] with a trailing ones column per
    head, so the O matmul lhsT=[v|ones] gives row 64 = softmax denominator and
    rows 0..63 = unnormalized o^T in one PSUM accumulation chain.
  - head PAIRS: the two heads' S matmuls sit at base partitions 0/64 (distinct
    PE row groups) and run concurrently.

v3 perf structure (baseline 364us -> v2 353.5us -> this):
  - unified big-PSUM pool (bufs=3, 6 banks) for v/qk/S/proj accumulators: the
    attention S ring is 3 deep, so the inner loop is throughput-bound on the
    PE instead of latency-bound on exp PSUM release.
  - attention emitted in 4j GROUPS [S,S,S, O-batch-of-8, S] with a pend queue
    that rolls ACROSS i4 blocks (lag ~8 chunks): long same-shape O runs cut
    the ~100ns 64<->128-row-mode transition tax from ~17 to ~8 per block and
    remove the block-boundary pipeline drains; the batch sits right before
    the st-ring-gated 4th S so the PE chews O work during the exp wait.
  - host passes all inputs PRE-REARRANGED partition-major ([128, ...] C-order)
    so every load DMA has 2-16KB contiguous lines instead of the 256B-1KB
    strided runs of the (c p)->p c rearrange (those capped HBM at ~240GB/s
    and pushed the first v matmul to 14us).  wv + the first xT token block
    land first; the first v chain starts ~4us earlier.
  - bias algebra: v-bias folded into the host-side output bias
    (b_eff = b_proj + b_v @ W_proj, exact because softmax rows sum to 1);
    k-bias dropped entirely (softmax is invariant to per-query shifts).
  - exp split: ScalarE exact exp (A) / VectorE Schraudolph bit-trick (D) in
    strict alternation ADAD...AA (9A/7D) so consecutive chunks land on
    different engines; ~2% rms on D chunks, total rel err ~1.5e-2 vs 2e-2.
  - PE warm-up matmuls during the initial DMA wait (HAM clock gate).
"""

import os
from contextlib import ExitStack

import numpy as np
import ml_dtypes

import concourse.bass as bass
import concourse.tile as tile
from concourse import bacc, mybir
from concourse.bass import ds, ts
from concourse.bass_utils import run_bass_kernel_spmd

try:  # without the NTFF hook module, a stray BASS_TRACE=1 would crash the run
    from antenv.axon_hooks import get_axon_ntff_profile_hook  # noqa: F401
except ImportError:
    os.environ.setdefault("BASS_NEVER_TRACE", "1")

BF16 = mybir.dt.bfloat16
F32 = mybir.dt.float32
NP_BF16 = ml_dtypes.bfloat16

LOG2E = float(np.log2(np.e))
SHIFT = 0.0  # bf16 exp needs no range shift (max logit 8.08 -> e^8 fits)
A16 = (2.0**7) * LOG2E / 8.0
B16 = 127.0 * 2**7 - (2.0**7) * LOG2E * SHIFT - 5.5
I16 = mybir.dt.int16
# per-i4-block chunk pattern: A = ScalarE exact exp, D = VectorE Schraudolph
PATTERN = "AADADADADADADAAA"

B, N, C = 4, 2048, 1024
H, D = 16, 64
HPC = 8            # heads per core
CD = HPC * D       # 512 local qkv dims per core
E = D + 1          # 65: 64 v dims + ones column

LAST_RESULTS = None  # stash for test harness (exec_time_ns, trace paths)


def _build_program():
    nc = bacc.Bacc("TRN2", target_bir_lowering=False, debug=False)

    # all inputs partition-major, pre-rearranged on host for contiguous DMA
    xT_d = nc.dram_tensor("xT", [128, 8, N], BF16, kind="ExternalInput").ap()
    wqk_d = nc.dram_tensor("wqk", [128, 8, 2 * CD], BF16, kind="ExternalInput").ap()
    wv_d = nc.dram_tensor("wv", [128, 8, CD], BF16, kind="ExternalInput").ap()
    bq_d = nc.dram_tensor("bq", [128, 4], F32, kind="ExternalInput").ap()
    wp_d = nc.dram_tensor("wp", [128, 4, C], BF16, kind="ExternalInput").ap()
    out_d = nc.dram_tensor("out", [N, C], F32, kind="ExternalOutput").ap()

    with tile.TileContext(nc) as tc, ExitStack() as ctx:
        singles = ctx.enter_context(tc.tile_pool(name="singles", bufs=1))
        # one big-PSUM pool: 3 x [128,1024]f32 slots (6 banks) shared by the
        # warmup/v/qk/S/proj accumulators -> 3-deep S ring in attention
        ps_pool = ctx.enter_context(tc.tile_pool(name="ps", bufs=3, space="PSUM"))
        ot_pool = ctx.enter_context(tc.tile_pool(name="ot", bufs=2, space="PSUM"))
        exp_pool = ctx.enter_context(tc.tile_pool(name="expp", bufs=13))
        misc = ctx.enter_context(tc.tile_pool(name="misc", bufs=4))
        ob_pool = ctx.enter_context(tc.tile_pool(name="ob", bufs=4))

        # Persistent SBUF tensors, chunk-major: [partition, chunk, free].
        xT_sb = singles.tile([128, 8, N], BF16)        # x^T   [c, token]
        wqk_sb = singles.tile([128, 8, 2 * CD], BF16)  # W_qk  [c, m]
        wv_sb = singles.tile([128, 8, CD], BF16)       # W_v   [c, n]
        bq_sb = singles.tile([128, 4], F32)
        wp_sb = singles.tile([128, 4, C], BF16)        # W_proj [hd, n]
        qkT_sb = singles.tile([128, 8, N], BF16)       # chunks 0..3 = q, 4..7 = k
        v_sb = singles.tile([128, 16, HPC * E], BF16)  # [token-in-chunk, tchunk, h*(64+1)]
        oT_sb = singles.tile([128, 4, N], BF16)        # o^T, proj lhsT layout
        nbias_sb = singles.tile([128, 1], F32)         # -SHIFT bias for ACT exp

        # DMA priority order = emission order, on TWO HWDGE rings in parallel:
        # the scalar ring issues the weights (wv first -- the v phase blocks
        # on it), the sync ring streams xT in 256-token pieces (2KB
        # contiguous per partition) in v-chunk consumption order.
        nc.scalar.dma_start(wv_sb, wv_d)
        nc.scalar.dma_start(wqk_sb, wqk_d)
        nc.scalar.dma_start(bq_sb, bq_d)
        nc.scalar.dma_start(wp_sb, wp_d)
        # xT pieces: contiguous 1KB runs on BOTH the DRAM and SBUF side,
        # token-quarter-major so the first v chunks unblock earliest
        for tq in range(4):
            for kc in range(8):
                nc.sync.dma_start(
                    xT_sb[:, kc, ds(tq * 512, 512)], xT_d[:, kc, ds(tq * 512, 512)]
                )
        nc.vector.memset(nbias_sb, -SHIFT)

        # dummy exp: pulls the ~2.7us ACT table load into the DMA/v ramp
        # instead of the first real activation on block (0,0)'s critical path
        warm = misc.tile([128, 1], BF16, tag="warm")
        nc.scalar.activation(
            warm, nbias_sb, mybir.ActivationFunctionType.Exp
        )

        # PE warm-up: matmuls on a memset tile during the initial DMA wait
        # flip the HAM clock gate to 8/8 before the first v chunk.
        warm_in = misc.tile([128, 256], BF16, tag="warmin")
        nc.vector.memset(warm_in, 0.0)
        warm_ps = ps_pool.tile([128, 1024], F32, tag="st", name="warmps")
        for _ in range(26):
            nc.tensor.matmul(
                warm_ps[:, 0:256], warm_in[:, 0:128], warm_in, start=True, stop=True
            )

        # ones columns for all 16 v chunks in one strided memset
        nc.vector.memset(
            v_sb.rearrange("p t (h e) -> p t h e", e=E)[:, :, :, D : D + 1], 1.0
        )

        # v natural layout; bias is folded into the host-side output bias.
        def emit_v_chunk(t):
            ps = ps_pool.tile([128, 512], F32, tag="st", name="vps")
            for kc in range(8):
                nc.tensor.matmul(
                    ps,
                    xT_sb[:, kc, ts(t, 128)],
                    wv_sb[:, kc, :],
                    start=(kc == 0),
                    stop=(kc == 7),
                )
            vv = v_sb[:, t].rearrange("p (h e) -> p h e", e=E)
            nc.vector.tensor_copy(vv[:, :, 0:D], ps.rearrange("p (h d) -> p h d", d=D))

        for t in range(16):
            emit_v_chunk(t)

        # qk chunk m: q chunks (m<4) get the q bias on evacuation; k chunks
        # (m>=4) need no bias (softmax is invariant to per-query shifts).
        def emit_qk_chunk(m):
            for i4q in range(4):
                ps = ps_pool.tile([128, 512], F32, tag="st", name="qkps")
                for kc in range(8):
                    nc.tensor.matmul(
                        ps,
                        wqk_sb[:, kc, ts(m, 128)],
                        xT_sb[:, kc, ds(i4q * 512, 512)],
                        start=(kc == 0),
                        stop=(kc == 7),
                    )
                if m < 4:
                    nc.vector.tensor_scalar_add(
                        qkT_sb[:, m, ds(i4q * 512, 512)], ps, bq_sb[:, ds(m, 1)]
                    )
                else:
                    nc.vector.tensor_copy(qkT_sb[:, m, ds(i4q * 512, 512)], ps)

        def emit_proj(t):
            for nh in range(2):
                pp = ps_pool.tile([128, 512], F32, tag="st", name="pp")
                for hc in range(4):
                    nc.tensor.matmul(
                        pp,
                        oT_sb[:, hc, ts(t, 128)],
                        wp_sb[:, hc, ds(nh * 512, 512)],
                        start=(hc == 0),
                        stop=(hc == 3),
                    )
                ob = ob_pool.tile([128, 512], F32)
                nc.vector.tensor_copy(ob, pp)
                nc.sync.dma_start(out_d[ts(t, 128), ds(nh * 512, 512)], ob)

        # Phase 2: per head PAIR. The two heads' S matmuls target different PE
        # row groups (base partitions 0 / 64) so back-to-back issue runs them
        # concurrently; both write one [128, 1024] st tile and share one exp.
        for p in range(4):
            qA, qB = qkT_sb[0:64, p], qkT_sb[64:128, p]
            kA, kB = qkT_sb[0:64, 4 + p], qkT_sb[64:128, 4 + p]
            hA, hB = 2 * p, 2 * p + 1
            emit_qk_chunk(p)
            emit_qk_chunk(4 + p)

            def emit_evac(i4, ot_list):
                i0 = i4 * 512
                for hp_, ot in ot_list:
                    # Copy the whole accumulator out first: frees the PSUM
                    # slot fast; same DVE cost as one row (partitions are
                    # parallel).  MUST stay on DVE: on ScalarE it head-of-line
                    # blocks the exp stream (waits for the whole O chain).
                    otc = misc.tile([65, 512], F32, tag="otc")
                    nc.vector.tensor_copy(otc, ot)
                    # Softmax denominators: lane-scatter so reciprocal runs on
                    # 128 lanes x 4 elems instead of 1 lane x 512
                    s_t = misc.tile([128, 4], F32, tag="sct")
                    nc.sync.dma_start(s_t, otc[64:65])
                    r_t = misc.tile([128, 4], F32, tag="rct")
                    nc.vector.reciprocal(r_t, s_t)
                    rec0 = misc.tile([1, 512], F32, tag="rec0")
                    nc.sync.dma_start(rec0, r_t)
                    recb = misc.tile([64, 512], F32, tag="recb")
                    nc.gpsimd.partition_broadcast(recb, rec0)
                    tmp = misc.tile([64, 512], BF16, tag="tmp")
                    nc.vector.tensor_mul(tmp, otc[0:64], recb)
                    nc.sync.dma_start(oT_sb[hp_ : hp_ + 64, p, ds(i0, 512)], tmp)

            for i4 in range(4):
                i0 = i4 * 512
                otA = ot_pool.tile([65, 512], F32, tag="ot")
                otB = ot_pool.tile([65, 512], F32, tag="ot")
                pend = []

                def emit_o(batch):
                    # uniform run of 4 O matmuls (2 chunks x 2 heads): longer
                    # same-shape runs halve the PE mode-switch tax
                    for ot, h in ((otA, hA), (otB, hB)):
                        for j, ex in batch:
                            vvj = v_sb[:, j].rearrange("p (h e) -> p h e", e=E)
                            nc.tensor.matmul(
                                ot,
                                vvj[:, h],
                                ex[:, ds(512 * (h % 2), 512)],
                                start=(j == 0),
                                stop=(j == 15),
                            )

                for j in range(16):
                    st = ps_pool.tile([128, 1024], F32, tag="st", name="st")
                    nc.tensor.matmul(
                        st[:, 0:512], kA[:, ts(j, 128)], qA[:, ds(i0, 512)],
                        start=True, stop=True,
                    )
                    nc.tensor.matmul(
                        st[:, 512:1024], kB[:, ts(j, 128)], qB[:, ds(i0, 512)],
                        start=True, stop=True,
                    )
                    ex = exp_pool.tile([128, 1024], BF16)
                    if PATTERN[j] == "A":
                        nc.scalar.activation(
                            ex, st, mybir.ActivationFunctionType.Exp,
                            scale=float(D) ** -0.5,
                        )
                    else:
                        nc.vector.tensor_scalar(
                            ex.bitcast(I16), st, A16, B16,
                            mybir.AluOpType.mult, mybir.AluOpType.add,
                        )
                    pend.append((j, ex))
                    # lag-3: emit O only for exps issued >=3 chunks ago, so
                    # the PE never head-of-line blocks on a fresh exp
                    if len(pend) >= 5:
                        emit_o([pend.pop(0), pend.pop(0)])
                while pend:
                    emit_o([pend.pop(0), pend.pop(0)] if len(pend) >= 2 else [pend.pop(0)])
                emit_evac(i4, ((0, otA), (64, otB)))

        for t in range(16):
            emit_proj(t)

    nc.compile()
    return nc


_PROGRAM = None


def _relayout_pcm(w, nchunk):
    # [nchunk*128, M] -> [128, nchunk, M] partition-major contiguous
    m = w.shape[1]
    return np.ascontiguousarray(
        w.reshape(nchunk, 128, m).transpose(1, 0, 2)
    ).astype(NP_BF16)


def kernel(x, W_qkv, b_qkv, W_proj, b_proj):
    global _PROGRAM, LAST_RESULTS
    x = np.asarray(x, dtype=np.float32)
    W_qkv = np.asarray(W_qkv, dtype=np.float32)
    b_qkv = np.asarray(b_qkv, dtype=np.float32)
    W_proj = np.asarray(W_proj, dtype=np.float32)
    b_proj = np.asarray(b_proj, dtype=np.float32)

    if _PROGRAM is None:
        _PROGRAM = _build_program()
    nc = _PROGRAM

    # effective output bias: v-bias flows through softmax (rows sum to 1)
    # straight into the projection
    b_eff = b_proj + b_qkv[2 * C :] @ W_proj

    in_maps = []
    for core in range(8):
        b, hg = core // 2, core % 2
        h0 = hg * HPC
        sl = slice(h0 * D, h0 * D + CD)
        wq = W_qkv[:, 0 * C :][:, sl]
        wk = W_qkv[:, 1 * C :][:, sl]
        wv = W_qkv[:, 2 * C :][:, sl]
        bq = b_qkv[0 * C :][sl]
        # xT: [C, N] -> [128p, 8kc, 2048t] partition-major contiguous
        xT = np.ascontiguousarray(
            x[b].T.reshape(8, 128, N).transpose(1, 0, 2)
        ).astype(NP_BF16)
        in_maps.append(
            {
                "xT": xT,
                "wqk": _relayout_pcm(np.concatenate([wq, wk], axis=1), 8),
                "wv": _relayout_pcm(np.ascontiguousarray(wv), 8),
                "bq": bq.reshape(4, 128).T.astype(np.float32).copy(),
                "wp": _relayout_pcm(np.ascontiguousarray(W_proj[sl, :]), 4),
            }
        )

    res = run_bass_kernel_spmd(nc, in_maps, list(range(8)))
    LAST_RESULTS = res
    out = np.empty((B, N, C), dtype=np.float32)
    for b in range(B):
        out[b] = (
            res.results[2 * b]["out"].astype(np.float32)
            + res.results[2 * b + 1]["out"].astype(np.float32)
            + b_eff[None, :]
        )
    return out
